# revision 2
# baseline (speedup 1.0000x reference)
"""AVWGCN (adaptive graph conv) Trainium2 kernel.

Math (K=3 Chebyshev, S = softmax_rows(relu(A @ E))):
  out_b = x_b@(W0-W2) + bias + S@(x_b@W1 + 2*S@(x_b@W2))

We never materialize normalized S. Instead P = exp(relu(r)) (via
max(exp(r),1)) with r = A@E, d = rowsum(P), and the 1/d row scaling is
applied on PSUM eviction after each P-matmul.

Sharding: data-parallel over batch B (8 cores x 8 batches). P^T build is
replicated on every core; P^T is spilled to device DRAM in a strip-ordered
layout ([pair, mt, p, 256] bf16, 512B contiguous runs) and streamed back as
lhsT strips for the two aggregation stages.

Per-core layouts:
  xaug  [8, 65, N] bf16   host-prepped x^T per batch with a ones row
  wcat  [65, 192] bf16    [[W0-W2; bias], [W1; 0], [2*W2; 0]]
  yall  SBUF [128, NT*1536] = (mt, b, w, c) mix results
  u     SBUF [128, NT*512] = stage-1 output (rhs of stage 2)
  out   [N, 8*64] f32     contiguous eviction layout; host reshapes
"""

import os
import sys

for _p in ("/root/.axon_site", "/root/.axon_site/_ro/trn_rl_repo",
           "/root/.axon_site/_ro/pypackages"):
    if os.path.isdir(_p) and _p not in sys.path:
        sys.path.append(_p)

import numpy as np
import ml_dtypes

import concourse.bass as bass
import concourse.mybir as mybir
import concourse.tile as tile
from concourse import bacc
from concourse.bass_utils import run_bass_kernel_spmd

BF16 = mybir.dt.bfloat16
F32 = mybir.dt.float32
NP_BF16 = ml_dtypes.bfloat16

N = 4096
E = 16
CI = 64
CO = 64
BLOC = 8
NCORES = 8


def build_nc(n=N, bloc=BLOC):
    nt = n // 128          # node tiles / m tiles
    nch = n // 512         # 512-wide chunks for the P^T build
    npair = n // 256       # 256-wide strip column pairs
    bc = bloc * CO         # free width of the stage matmuls (512)
    mixw = 3 * CO          # 192 columns of the mix matmul

    nc = bacc.Bacc(None)
    xaug_d = nc.declare_dram_parameter("xaug", [bloc, CI + 1, n], BF16, isOutput=False)
    embt_d = nc.declare_dram_parameter("embt", [E, n], BF16, isOutput=False)
    at_d = nc.declare_dram_parameter("at", [E, n], BF16, isOutput=False)
    wcat_d = nc.declare_dram_parameter("wcat", [CI + 1, mixw], BF16, isOutput=False)
    out_d = nc.declare_dram_parameter("out", [n, bc], F32, isOutput=True)

    Exp = mybir.ActivationFunctionType.Exp
    mult = mybir.AluOpType.mult
    add = mybir.AluOpType.add

    with tile.TileContext(nc) as tc:
        with (
            tc.tile_pool(name="dram", bufs=1, space="DRAM") as dpool,
            tc.tile_pool(name="const", bufs=1) as cpool,
            tc.tile_pool(name="big", bufs=1) as big,
            tc.tile_pool(name="ps", bufs=2, space="PSUM") as ps,
        ):
            # P^T spill, strip-ordered: [pair, mt, p, nw]
            ptd = dpool.tile([npair * nt * 128, 256], BF16)
            ptd_v = ptd.rearrange("(pair mt p) nw -> pair mt p nw", mt=nt, p=128)
            dbounce = dpool.tile([1, n], F32)

            wcat_sb = cpool.tile([CI + 1, mixw], BF16)
            nc.sync.dma_start(wcat_sb[:], wcat_d[:])
            ones_sb = cpool.tile([128, 1], BF16)
            nc.vector.memset(ones_sb[:], 1.0)
            d_row = cpool.tile([1, n], F32)
            invd = cpool.tile([128, nt], F32)

            yall = big.tile([128, nt * bloc * mixw], BF16)
            yall_v = yall.rearrange(
                "p (mt b w c) -> p mt b w c", mt=nt, b=bloc, w=3, c=CO
            )
            u = big.tile([128, nt * bc], BF16)
            u_v = u.rearrange("p (mt b c) -> p mt b c", mt=nt, b=bloc, c=CO)

            # ---- Phase P: build P^T = max(exp(A@E)^T, 1) -> DRAM; d = colsums
            with tc.tile_pool(name="bld", bufs=1) as bld:
                embt_sb = bld.tile([E, n], BF16)
                nc.sync.dma_start(embt_sb[:], embt_d[:])
                at_sb = bld.tile([E, n], BF16)
                nc.sync.dma_start(at_sb[:], at_d[:])
                for ch in range(nch):
                    d_ps = ps.tile([1, 512], F32, tag="d", bufs=2)
                    for mt in range(nt):
                        r_ps = ps.tile([128, 512], F32, tag="r")
                        nc.tensor.matmul(
                            r_ps[:],
                            lhsT=embt_sb[:, mt * 128:(mt + 1) * 128],
                            rhs=at_sb[:, ch * 512:(ch + 1) * 512],
                            start=True, stop=True,
                        )
                        pt = bld.tile([128, 512], BF16, tag="pt", bufs=3)
                        nc.scalar.activation(pt[:], r_ps[:], Exp)
                        nc.vector.tensor_scalar_max(pt[:], pt[:], 1.0)
                        nc.tensor.matmul(
                            d_ps[:], lhsT=ones_sb[:], rhs=pt[:],
                            start=(mt == 0), stop=(mt == nt - 1),
                        )
                        nc.scalar.dma_start(
                            ptd_v[2 * ch:2 * ch + 2, mt].rearrange(
                                "pair p nw -> p pair nw"),
                            pt.rearrange("p (pair nw) -> p pair nw", pair=2),
                        )
                    nc.vector.tensor_copy(d_row[:, ch * 512:(ch + 1) * 512], d_ps[:])

                # d: [1, n] -> DRAM bounce -> [128, nt] column layout -> 1/d
                nc.sync.dma_start(dbounce[:], d_row[:])
                d_col = bld.tile([128, nt], F32)
                nc.sync.dma_start(
                    d_col[:], dbounce.rearrange("one (t p) -> p (one t)", p=128)
                )
                nc.vector.reciprocal(invd[:], d_col[:])

            # ---- Phase Y: mix Y = [x,1] @ wcat per (b, mt) -> yall
            with tc.tile_pool(name="mix", bufs=1) as mix:
                for b in range(bloc):
                    xa = mix.tile([CI + 1, n], BF16, tag="xa", bufs=2)
                    nc.gpsimd.dma_start(xa[:], xaug_d[b])
                    for mt in range(nt):
                        y_ps = ps.tile([128, mixw], F32, tag="y")
                        nc.tensor.matmul(
                            y_ps[:],
                            lhsT=xa[:, mt * 128:(mt + 1) * 128],
                            rhs=wcat_sb[:],
                            start=True, stop=True,
                        )
                        nc.scalar.copy(
                            yall[:, mt * (bloc * mixw) + b * mixw:
                                 mt * (bloc * mixw) + (b + 1) * mixw],
                            y_ps[:],
                        )

            # ---- Stages: u = invd*(P@Y2)+Y1 ; out = invd*(P@u)+Y0
            with tc.tile_pool(name="stg", bufs=1) as stg:
                for stage in (1, 2):
                    for pair in range(npair):
                        strip = stg.tile([128, nt * 256], BF16, tag="strip", bufs=2)
                        nc.sync.dma_start(
                            strip.rearrange("p (mt nw) -> p mt nw", nw=256),
                            ptd_v[pair].rearrange("mt p nw -> p mt nw"),
                        )
                        for sub in range(2):
                            ntile = pair * 2 + sub
                            z_ps = ps.tile([128, bc], F32, tag="z")
                            for mt in range(nt):
                                rhs = (yall_v[:, mt, :, 2, :] if stage == 1
                                       else u_v[:, mt])
                                nc.tensor.matmul(
                                    z_ps[:],
                                    lhsT=strip[:, mt * 256 + sub * 128:
                                               mt * 256 + sub * 128 + 128],
                                    rhs=rhs,
                                    start=(mt == 0), stop=(mt == nt - 1),
                                )
                            z_v = z_ps.rearrange("p (b c) -> p b c", b=bloc)
                            if stage == 1:
                                nc.vector.scalar_tensor_tensor(
                                    out=u_v[:, ntile],
                                    in0=z_v[:],
                                    scalar=invd[:, ntile:ntile + 1],
                                    in1=yall_v[:, ntile, :, 1, :],
                                    op0=mult, op1=add,
                                )
                            else:
                                o = stg.tile([128, bc], F32, tag="o", bufs=2)
                                nc.vector.scalar_tensor_tensor(
                                    out=o.rearrange("p (b c) -> p b c", b=bloc),
                                    in0=z_v[:],
                                    scalar=invd[:, ntile:ntile + 1],
                                    in1=yall_v[:, ntile, :, 0, :],
                                    op0=mult, op1=add,
                                )
                                nc.gpsimd.dma_start(
                                    out_d[ntile * 128:(ntile + 1) * 128, :], o[:]
                                )
    nc.finalize()
    return nc


_NC_CACHE = {}


def _get_nc(n=N, bloc=BLOC):
    key = (n, bloc)
    if key not in _NC_CACHE:
        _NC_CACHE[key] = build_nc(n, bloc)
    return _NC_CACHE[key]


def make_in_maps(x, adj_matrix, adj_embeddings, weights, bias, n=N, bloc=BLOC):
    ncores = x.shape[0] // bloc
    w0, w1, w2 = np.asarray(weights, np.float32)
    wc = np.zeros((CI + 1, 3 * CO), np.float32)
    wc[:CI, :CO] = w0 - w2
    wc[CI, :CO] = np.asarray(bias, np.float32)
    wc[:CI, CO:2 * CO] = w1
    wc[:CI, 2 * CO:] = 2.0 * w2

    at = np.ascontiguousarray(np.asarray(adj_matrix, np.float32).T).astype(NP_BF16)
    embt = np.ascontiguousarray(np.asarray(adj_embeddings, np.float32)).astype(NP_BF16)
    wcat = wc.astype(NP_BF16)

    xaug = np.empty((x.shape[0], CI + 1, n), np.float32)
    xaug[:, :CI, :] = np.asarray(x, np.float32).transpose(0, 2, 1)
    xaug[:, CI, :] = 1.0
    xaug = xaug.astype(NP_BF16)

    return [
        {
            "xaug": np.ascontiguousarray(xaug[c * bloc:(c + 1) * bloc]),
            "embt": embt,
            "at": at,
            "wcat": wcat,
        }
        for c in range(ncores)
    ]


def assemble_out(results, n=N, bloc=BLOC):
    """results: list of per-core dicts with 'out' [n, bloc*CO] -> [B, n, CO]."""
    outs = []
    for r in results:
        o = np.asarray(r["out"]).reshape(n, bloc, CO).transpose(1, 0, 2)
        outs.append(o)
    return np.ascontiguousarray(np.concatenate(outs, axis=0), dtype=np.float32)


def kernel(x, adj_matrix, adj_embeddings, weights, bias):
    x = np.asarray(x)
    in_maps = make_in_maps(x, adj_matrix, adj_embeddings, weights, bias)
    nc = _get_nc()
    res = run_bass_kernel_spmd(nc, in_maps, core_ids=list(range(NCORES)))
    return assemble_out(res.results)
